# revision 17
# baseline (speedup 1.0000x reference)
"""CrossTransformer kernel for Trainium2, data-parallel over batch across 8 cores.

Math per batch b (B=32, N=25, C=512, H=W=14, DK=DV=128):
  qq = Wqk @ Q    [128, 196]      qv = Wv @ Q     [128, 196]
  K  = Wqk @ S    [128, 4900]     V  = Wv @ S     [128, 4900]
  simT[nij, hw] = K^T @ qq        (computed directly in transposed layout)
  E = exp(simT)                   (no max subtraction; |sim| <~ 60 is safe in fp32)
  ctx_raw[hw, v+1] = sum_nij E[nij, hw]^T @ [V^T | 1]   (ones column -> softmax denom)
  ctx = ctx_raw[:, :128] / ctx_raw[:, 128:129]
  partial += sum((qv^T - ctx)^2)
Output per core: scalar partial sum over its 4 batches; host sums and divides by H*W.

S is streamed with one DMA per (batch, c-chunk, n-half): 8 transfers of
~1.2-1.3 MB per batch (784B contiguous runs), instead of many ~100-250KB
pieces - the descriptor-generation path (HWDGE, ~625ns/DMA) serializes all
DMAs, so transfer count, not just bytes, bounds the kernel.
"""

import os
import sys

sys.path.insert(0, "/opt/trn_rl_repo")

import numpy as np

import concourse.bass as bass
import concourse.bacc as bacc
import concourse.mybir as mybir
import concourse.tile as tile
from concourse.bass_utils import run_bass_kernel_spmd
from concourse.masks import make_identity

F32 = mybir.dt.float32
F32R = mybir.dt.float32r
BF16 = mybir.dt.bfloat16

B_PER_CORE = 4
N_SUP = 25
C = 512
HW = 196
NIJ = N_SUP * HW  # 4900
DK = 128
NCH = (NIJ + 127) // 128  # 39 chunks of <=128 along nij
CCH = C // 128  # 4 c-chunks
NPIECES = [(0, 6), (6, 12), (12, 18), (18, 23), (23, N_SUP)]  # one DMA per (b, cc, piece)
SMAX = max(n1 - n0 for n0, n1 in NPIECES) * HW  # 1176: widest piece tile


def _piece_chunks():
    """Assign each 128-wide nij chunk to the first piece whose data covers it."""
    out = [[] for _ in NPIECES]
    ends = [n1 * HW for _, n1 in NPIECES]
    for j in range(NCH):
        hi = min((j + 1) * 128, NIJ)
        p = next(i for i, e in enumerate(ends) if hi <= e)
        out[p].append(j)
    return out


PIECE_CHUNKS = _piece_chunks()


def _proj_widths(w):
    """Split w into <=512-wide even tiles >=256 (fp32r full-rate, ISA-legal)."""
    nt = (w + 511) // 512
    base = (w // (2 * nt)) * 2
    ws = [base + 2] * ((w - nt * base) // 2)
    ws += [base] * (nt - len(ws))
    return ws


def build_bass():
    nc = bacc.Bacc(
        "TRN2", target_bir_lowering=False, debug=False, enable_asserts=False
    )
    q_d = nc.dram_tensor("q", [B_PER_CORE, C, HW], F32, kind="ExternalInput").ap()
    s_d = nc.dram_tensor(
        "s", [B_PER_CORE, N_SUP, C, HW], F32, kind="ExternalInput"
    ).ap()
    wqk_d = nc.dram_tensor("wqk", [DK, C], F32, kind="ExternalInput").ap()
    wv_d = nc.dram_tensor("wv", [DK, C], F32, kind="ExternalInput").ap()
    out_d = nc.dram_tensor("out", [1, 1], F32, kind="ExternalOutput").ap()

    with tile.TileContext(nc) as tc:
        with (
            tc.tile_pool(name="const", bufs=1) as const,
            tc.tile_pool(name="spool", bufs=16) as spool,
            tc.tile_pool(name="kvbf", bufs=4) as kvbf,
            tc.tile_pool(name="vt1p", bufs=24) as vt1p,
            tc.tile_pool(name="etp", bufs=24) as etp,
            tc.tile_pool(name="small", bufs=4) as small,
            tc.tile_pool(name="ps_proj", bufs=2, space="PSUM") as ps_proj,
            tc.tile_pool(name="ps_sim", bufs=2, space="PSUM") as ps_sim,
            tc.tile_pool(name="ps_vt", bufs=2, space="PSUM") as ps_vt,
            tc.tile_pool(name="ps_ctx", bufs=1, space="PSUM") as ps_ctx,
        ):
            # ---- constants / weights ----
            id_f32 = const.tile([128, 128], F32, tag="id_f32")
            make_identity(nc, id_f32)
            id_bf = const.tile([128, 128], BF16, tag="id_bf")
            make_identity(nc, id_bf)

            wqk_sb = const.tile([128, C], F32, tag="wqk_sb")
            nc.sync.dma_start(out=wqk_sb, in_=wqk_d)
            wv_sb = const.tile([128, C], F32, tag="wv_sb")
            nc.sync.dma_start(out=wv_sb, in_=wv_d)

            wqkT = []
            wvT = []
            for cc in range(CCH):
                for (src, dstl, nm) in ((wqk_sb, wqkT, "qk"), (wv_sb, wvT, "v")):
                    pt = ps_vt.tile([128, 128], F32, tag="ps_vt")
                    nc.tensor.transpose(pt, src[:, cc * 128 : (cc + 1) * 128], id_f32)
                    wt = const.tile([128, 128], F32R, tag=f"w{nm}T{cc}")
                    nc.vector.tensor_copy(wt, pt)
                    dstl.append(wt)

            # ---- query load + projections (all 4 batches at once) ----
            qsb = []
            for cc in range(CCH):
                qt = spool.tile([128, B_PER_CORE * HW], F32R, tag="s_t")
                src = q_d[:, cc * 128 : (cc + 1) * 128, :].rearrange(
                    "b c ij -> c b ij"
                ).bitcast(F32R)
                nc.sync.dma_start(
                    out=qt.rearrange("p (b ij) -> p b ij", b=B_PER_CORE), in_=src
                )
                qsb.append(qt)

            qq_bf = const.tile([128, B_PER_CORE * HW], BF16, tag="qq_bf")
            qv_sb = const.tile([128, B_PER_CORE * HW], F32, tag="qv_sb")
            for wT, dst in ((wqkT, qq_bf), (wvT, qv_sb)):
                for half in range(2):
                    hw0 = half * 392
                    pq = ps_proj.tile([128, 512], F32, tag="ps_proj")
                    for cc in range(CCH):
                        nc.tensor.matmul(
                            pq[:, :392],
                            lhsT=wT[cc],
                            rhs=qsb[cc][:, hw0 : hw0 + 392],
                            start=(cc == 0),
                            stop=(cc == CCH - 1),
                        )
                    nc.vector.tensor_copy(dst[:, hw0 : hw0 + 392], pq[:, :392])

            # qv^T per (b, hw-chunk): [hw<=128, 128] fp32 — matches ctx layout
            qvT = {}
            for b in range(B_PER_CORE):
                for h in range(2):
                    hww = 128 if h == 0 else HW - 128
                    pt = ps_vt.tile([128, 128], F32, tag="ps_vt")
                    nc.tensor.transpose(
                        pt[:hww, :],
                        qv_sb[:, b * HW + h * 128 : b * HW + h * 128 + hww],
                        id_f32,
                    )
                    qt = const.tile([128, 128], F32, tag=f"qvT{b}_{h}")
                    nc.vector.tensor_copy(qt[:hww, :], pt[:hww, :])
                    qvT[(b, h)] = qt

            partials = const.tile([128, 2 * B_PER_CORE], F32, tag="partials")
            nc.vector.memset(partials, 0.0)

            # ---- per-batch main pipeline, interleaved per n-piece ----
            # Per piece: DMA 4 c-slabs -> K/V proj -> sim+exp chunks ->
            # V^T transposes -> PV accumulation. This keeps the ACT exp chain
            # and PV under the DMA stream instead of serializing at batch end.
            for b in range(B_PER_CORE):
                k_bf = kvbf.tile([128, NIJ], BF16, tag="k_bf")
                v_bf = kvbf.tile([128, NIJ], BF16, tag="v_bf")
                # one PSUM bank per hw-half: start=True clears has_written
                # BANK-wide, so the two accumulation groups must not share.
                pc = [
                    ps_ctx.tile([128, 132], F32, tag="pc0", name="pc0"),
                    ps_ctx.tile([128, 132], F32, tag="pc1", name="pc1"),
                ]

                for p, (n0, n1) in enumerate(NPIECES):
                    nn = n1 - n0
                    w = nn * HW
                    off0 = n0 * HW
                    st = []
                    for cc in range(CCH):
                        s_t = spool.tile([128, SMAX], F32R, tag="s_t")
                        src = s_d[
                            b, n0:n1, cc * 128 : (cc + 1) * 128, :
                        ].rearrange("n c ij -> c n ij").bitcast(F32R)
                        nc.sync.dma_start(
                            out=s_t[:, 0:w].rearrange(
                                "p (n ij) -> p n ij", n=nn
                            ),
                            in_=src,
                        )
                        st.append(s_t)
                    t0 = 0
                    for tw in _proj_widths(w):
                        pk = ps_proj.tile([128, 512], F32, tag="ps_proj")
                        for cc in range(CCH):
                            nc.tensor.matmul(
                                pk[:, :tw],
                                lhsT=wqkT[cc],
                                rhs=st[cc][:, t0 : t0 + tw],
                                start=(cc == 0),
                                stop=(cc == CCH - 1),
                            )
                        nc.vector.tensor_copy(
                            k_bf[:, off0 + t0 : off0 + t0 + tw], pk[:, :tw]
                        )
                        pv = ps_proj.tile([128, 512], F32, tag="ps_proj")
                        for cc in range(CCH):
                            nc.tensor.matmul(
                                pv[:, :tw],
                                lhsT=wvT[cc],
                                rhs=st[cc][:, t0 : t0 + tw],
                                start=(cc == 0),
                                stop=(cc == CCH - 1),
                            )
                        nc.scalar.copy(
                            v_bf[:, off0 + t0 : off0 + t0 + tw], pv[:, :tw]
                        )
                        t0 += tw

                    # simT = K^T @ qq (bf16), exp -> E^T chunks of this piece
                    et = {}
                    for j in PIECE_CHUNKS[p]:
                        cw = min(128, NIJ - j * 128)
                        ps = ps_sim.tile([128, HW], F32, tag="ps_sim")
                        nc.tensor.matmul(
                            ps[:cw, :],
                            lhsT=k_bf[:, j * 128 : j * 128 + cw],
                            rhs=qq_bf[:, b * HW : (b + 1) * HW],
                            start=True,
                            stop=True,
                        )
                        e = etp.tile([128, HW], BF16, tag="et")
                        if cw < 128:
                            nc.vector.memset(e, 0.0)
                        nc.scalar.activation(
                            out=e[:cw, :],
                            in_=ps[:cw, :],
                            func=mybir.ActivationFunctionType.Exp,
                        )
                        et[j] = e

                    # V^T chunks (+ ones column) via PE transpose
                    vt1 = {}
                    for j in PIECE_CHUNKS[p]:
                        cw = min(128, NIJ - j * 128)
                        vt = vt1p.tile([128, 132], BF16, tag="vt1")
                        if cw < 128:
                            nc.vector.memset(vt, 0.0)
                        pt = ps_vt.tile([128, 128], BF16, tag="ps_vt")
                        nc.tensor.transpose(
                            pt[:cw, :], v_bf[:, j * 128 : j * 128 + cw], id_bf
                        )
                        nc.vector.tensor_copy(vt[:cw, 0:128], pt[:cw, :])
                        nc.vector.memset(vt[:, 128:132], 1.0)
                        vt1[j] = vt

                    # PV: accumulate ctx_raw for both hw-halves
                    for j in PIECE_CHUNKS[p]:
                        for h in range(2):
                            hww = 128 if h == 0 else HW - 128
                            nc.tensor.matmul(
                                pc[h][:hww, 0:132],
                                lhsT=et[j][:, h * 128 : h * 128 + hww],
                                rhs=vt1[j][:, 0:132],
                                start=(j == 0),
                                stop=(j == NCH - 1),
                            )

                for h in range(2):
                    hww = 128 if h == 0 else HW - 128
                    r = small.tile([128, 1], F32, tag="recip")
                    nc.vector.reciprocal(r[:hww], pc[h][:hww, 128:129])
                    ctx = small.tile([128, 128], F32, tag="ctx")
                    nc.vector.tensor_scalar_mul(
                        ctx[:hww, :], pc[h][:hww, 0:128], r[:hww]
                    )
                    d = small.tile([128, 128], F32, tag="diff")
                    nc.vector.tensor_sub(
                        d[:hww, :], qvT[(b, h)][:hww, :], ctx[:hww, :]
                    )
                    d2 = small.tile([128, 128], F32, tag="d2")
                    nc.vector.tensor_mul(d2[:hww, :], d[:hww, :], d[:hww, :])
                    nc.vector.reduce_sum(
                        partials[:hww, 2 * b + h : 2 * b + h + 1],
                        d2[:hww, :],
                        axis=mybir.AxisListType.X,
                    )

            # ---- final reduction to scalar ----
            tot = small.tile([128, 1], F32, tag="tot")
            nc.vector.reduce_sum(tot, partials, axis=mybir.AxisListType.X)
            ones = small.tile([128, 1], F32, tag="ones")
            nc.vector.memset(ones, 1.0)
            pf = ps_vt.tile([128, 128], F32, tag="ps_vt")
            nc.tensor.matmul(pf[0:1, 0:1], lhsT=tot, rhs=ones, start=True, stop=True)
            ob = small.tile([1, 1], F32, tag="ob")
            nc.vector.tensor_copy(ob, pf[0:1, 0:1])
            nc.sync.dma_start(out=out_d, in_=ob)

    nc.compile()
    return nc


_NC = None


def kernel(query_repr, supports_repr, W_qk, W_v):
    global _NC
    q = np.ascontiguousarray(np.asarray(query_repr, dtype=np.float32)).reshape(
        32, C, HW
    )
    s = np.ascontiguousarray(np.asarray(supports_repr, dtype=np.float32)).reshape(
        32, N_SUP, C, HW
    )
    wqk = np.ascontiguousarray(np.asarray(W_qk, dtype=np.float32))
    wv = np.ascontiguousarray(np.asarray(W_v, dtype=np.float32))

    if _NC is None:
        _NC = build_bass()

    in_maps = []
    for core in range(8):
        b0 = core * B_PER_CORE
        in_maps.append(
            {
                "q": np.ascontiguousarray(q[b0 : b0 + B_PER_CORE]),
                "s": np.ascontiguousarray(s[b0 : b0 + B_PER_CORE]),
                "wqk": wqk,
                "wv": wv,
            }
        )
    res = run_bass_kernel_spmd(
        _NC, in_maps, core_ids=list(range(8)),
        trace=bool(int(os.environ.get("KTRACE", "0"))),
    )
    total = sum(float(r["out"][0, 0]) for r in res.results) / float(HW)
    kernel._last_results = res
    return np.asarray(total, dtype=np.float32)


# revision 19
# speedup vs baseline: 1.0273x; 1.0273x over previous
"""CrossTransformer kernel for Trainium2, data-parallel over batch across 8 cores.

Math per batch b (B=32, N=25, C=512, H=W=14, DK=DV=128):
  qq = Wqk @ Q    [128, 196]      qv = Wv @ Q     [128, 196]
  K  = Wqk @ S    [128, 4900]     V  = Wv @ S     [128, 4900]
  simT[nij, hw] = K^T @ qq        (computed directly in transposed layout)
  E = exp(simT)                   (no max subtraction; |sim| <~ 60 is safe in fp32)
  ctx_raw[hw, v+1] = sum_nij E[nij, hw]^T @ [V^T | 1]   (ones column -> softmax denom)
  ctx = ctx_raw[:, :128] / ctx_raw[:, 128:129]
  partial += sum((qv^T - ctx)^2)
Output per core: scalar partial sum over its 4 batches; host sums and divides by H*W.

S is streamed with one DMA per (batch, c-chunk, n-half): 8 transfers of
~1.2-1.3 MB per batch (784B contiguous runs), instead of many ~100-250KB
pieces - the descriptor-generation path (HWDGE, ~625ns/DMA) serializes all
DMAs, so transfer count, not just bytes, bounds the kernel.
"""

import os
import sys

sys.path.insert(0, "/opt/trn_rl_repo")

import numpy as np

import concourse.bass as bass
import concourse.bacc as bacc
import concourse.mybir as mybir
import concourse.tile as tile
from concourse.bass_utils import run_bass_kernel_spmd
from concourse.masks import make_identity

F32 = mybir.dt.float32
F32R = mybir.dt.float32r
BF16 = mybir.dt.bfloat16

B_PER_CORE = 4
N_SUP = 25
C = 512
HW = 196
NIJ = N_SUP * HW  # 4900
DK = 128
NCH = (NIJ + 127) // 128  # 39 chunks of <=128 along nij
CCH = C // 128  # 4 c-chunks
NPIECES = [(0, 6), (6, 12), (12, 18), (18, 23), (23, N_SUP)]  # one DMA per (b, cc, piece)
SMAX = max(n1 - n0 for n0, n1 in NPIECES) * HW  # 1176: widest piece tile


def _piece_chunks():
    """Assign each 128-wide nij chunk to the first piece whose data covers it."""
    out = [[] for _ in NPIECES]
    ends = [n1 * HW for _, n1 in NPIECES]
    for j in range(NCH):
        hi = min((j + 1) * 128, NIJ)
        p = next(i for i, e in enumerate(ends) if hi <= e)
        out[p].append(j)
    return out


PIECE_CHUNKS = _piece_chunks()


def _proj_widths(w):
    """Split w into <=512-wide even tiles >=256 (fp32r full-rate, ISA-legal)."""
    nt = (w + 511) // 512
    base = (w // (2 * nt)) * 2
    ws = [base + 2] * ((w - nt * base) // 2)
    ws += [base] * (nt - len(ws))
    return ws


def build_bass():
    nc = bacc.Bacc(
        "TRN2", target_bir_lowering=False, debug=False, enable_asserts=False
    )
    q_d = nc.dram_tensor("q", [B_PER_CORE, C, HW], F32, kind="ExternalInput").ap()
    s_d = nc.dram_tensor(
        "s", [B_PER_CORE, N_SUP, C, HW], F32, kind="ExternalInput"
    ).ap()
    wqk_d = nc.dram_tensor("wqk", [DK, C], F32, kind="ExternalInput").ap()
    wv_d = nc.dram_tensor("wv", [DK, C], F32, kind="ExternalInput").ap()
    out_d = nc.dram_tensor("out", [1, 1], F32, kind="ExternalOutput").ap()

    with tile.TileContext(nc) as tc:
        with (
            tc.tile_pool(name="const", bufs=1) as const,
            tc.tile_pool(name="spool", bufs=22) as spool,
            tc.tile_pool(name="kvbf", bufs=2) as kvbf,
            tc.tile_pool(name="vt1p", bufs=16) as vt1p,
            tc.tile_pool(name="etp", bufs=16) as etp,
            tc.tile_pool(name="small", bufs=4) as small,
            tc.tile_pool(name="ps_proj", bufs=2, space="PSUM") as ps_proj,
            tc.tile_pool(name="ps_sim", bufs=2, space="PSUM") as ps_sim,
            tc.tile_pool(name="ps_vt", bufs=2, space="PSUM") as ps_vt,
            tc.tile_pool(name="ps_ctx", bufs=1, space="PSUM") as ps_ctx,
        ):
            # ---- constants / weights ----
            id_f32 = const.tile([128, 128], F32, tag="id_f32")
            make_identity(nc, id_f32)
            id_bf = const.tile([128, 128], BF16, tag="id_bf")
            make_identity(nc, id_bf)

            wqk_sb = const.tile([128, C], F32, tag="wqk_sb")
            nc.sync.dma_start(out=wqk_sb, in_=wqk_d)
            wv_sb = const.tile([128, C], F32, tag="wv_sb")
            nc.sync.dma_start(out=wv_sb, in_=wv_d)

            wqkT = []
            wvT = []
            for cc in range(CCH):
                for (src, dstl, nm) in ((wqk_sb, wqkT, "qk"), (wv_sb, wvT, "v")):
                    pt = ps_vt.tile([128, 128], F32, tag="ps_vt")
                    nc.tensor.transpose(pt, src[:, cc * 128 : (cc + 1) * 128], id_f32)
                    wt = const.tile([128, 128], F32R, tag=f"w{nm}T{cc}")
                    nc.vector.tensor_copy(wt, pt)
                    dstl.append(wt)

            # ---- query load + projections (all 4 batches at once) ----
            qsb = []
            for cc in range(CCH):
                qt = spool.tile([128, B_PER_CORE * HW], F32R, tag="s_t")
                src = q_d[:, cc * 128 : (cc + 1) * 128, :].rearrange(
                    "b c ij -> c b ij"
                ).bitcast(F32R)
                nc.sync.dma_start(
                    out=qt.rearrange("p (b ij) -> p b ij", b=B_PER_CORE), in_=src
                )
                qsb.append(qt)

            qq_bf = const.tile([128, B_PER_CORE * HW], BF16, tag="qq_bf")
            qv_sb = const.tile([128, B_PER_CORE * HW], F32, tag="qv_sb")
            for wT, dst in ((wqkT, qq_bf), (wvT, qv_sb)):
                for half in range(2):
                    hw0 = half * 392
                    pq = ps_proj.tile([128, 512], F32, tag="ps_proj")
                    for cc in range(CCH):
                        nc.tensor.matmul(
                            pq[:, :392],
                            lhsT=wT[cc],
                            rhs=qsb[cc][:, hw0 : hw0 + 392],
                            start=(cc == 0),
                            stop=(cc == CCH - 1),
                        )
                    nc.vector.tensor_copy(dst[:, hw0 : hw0 + 392], pq[:, :392])

            # qv^T per (b, hw-chunk): [hw<=128, 128] fp32 — matches ctx layout
            qvT = {}
            for b in range(B_PER_CORE):
                for h in range(2):
                    hww = 128 if h == 0 else HW - 128
                    pt = ps_vt.tile([128, 128], F32, tag="ps_vt")
                    nc.tensor.transpose(
                        pt[:hww, :],
                        qv_sb[:, b * HW + h * 128 : b * HW + h * 128 + hww],
                        id_f32,
                    )
                    qt = const.tile([128, 128], F32, tag=f"qvT{b}_{h}")
                    nc.vector.tensor_copy(qt[:hww, :], pt[:hww, :])
                    qvT[(b, h)] = qt

            partials = const.tile([128, 2 * B_PER_CORE], F32, tag="partials")
            nc.vector.memset(partials, 0.0)

            # ---- per-batch main pipeline, interleaved per n-piece ----
            # Per piece: DMA 4 c-slabs -> K/V proj -> sim+exp chunks ->
            # V^T transposes -> PV accumulation. This keeps the ACT exp chain
            # and PV under the DMA stream instead of serializing at batch end.
            for b in range(B_PER_CORE):
                k_bf = kvbf.tile([128, NIJ], BF16, tag="k_bf")
                v_bf = kvbf.tile([128, NIJ], BF16, tag="v_bf")
                # one PSUM bank per hw-half: start=True clears has_written
                # BANK-wide, so the two accumulation groups must not share.
                pc = [
                    ps_ctx.tile([128, 132], F32, tag="pc0", name="pc0"),
                    ps_ctx.tile([128, 132], F32, tag="pc1", name="pc1"),
                ]

                for p, (n0, n1) in enumerate(NPIECES):
                    nn = n1 - n0
                    w = nn * HW
                    off0 = n0 * HW
                    st = []
                    for cc in range(CCH):
                        s_t = spool.tile([128, SMAX], F32R, tag="s_t")
                        src = s_d[
                            b, n0:n1, cc * 128 : (cc + 1) * 128, :
                        ].rearrange("n c ij -> c n ij").bitcast(F32R)
                        nc.sync.dma_start(
                            out=s_t[:, 0:w].rearrange(
                                "p (n ij) -> p n ij", n=nn
                            ),
                            in_=src,
                        )
                        st.append(s_t)
                    # Per proj tile: V matmuls (copy on ACT, hidden under the
                    # K matmuls), K matmuls (copy on DVE), then sim+exp and
                    # V^T transposes for every chunk the copies now cover —
                    # this starts the serial ACT exp chain as early as
                    # possible instead of after the whole piece's projections.
                    et = {}
                    vt1 = {}
                    chunks = PIECE_CHUNKS[p]
                    ci = 0
                    t0 = 0
                    for tw in _proj_widths(w):
                        pv = ps_proj.tile([128, 512], F32, tag="ps_proj")
                        for cc in range(CCH):
                            nc.tensor.matmul(
                                pv[:, :tw],
                                lhsT=wvT[cc],
                                rhs=st[cc][:, t0 : t0 + tw],
                                start=(cc == 0),
                                stop=(cc == CCH - 1),
                            )
                        nc.scalar.copy(
                            v_bf[:, off0 + t0 : off0 + t0 + tw], pv[:, :tw]
                        )
                        pk = ps_proj.tile([128, 512], F32, tag="ps_proj")
                        for cc in range(CCH):
                            nc.tensor.matmul(
                                pk[:, :tw],
                                lhsT=wqkT[cc],
                                rhs=st[cc][:, t0 : t0 + tw],
                                start=(cc == 0),
                                stop=(cc == CCH - 1),
                            )
                        nc.vector.tensor_copy(
                            k_bf[:, off0 + t0 : off0 + t0 + tw], pk[:, :tw]
                        )
                        t0 += tw
                        covered = off0 + t0
                        nj = ci
                        while (
                            nj < len(chunks)
                            and min((chunks[nj] + 1) * 128, NIJ) <= covered
                        ):
                            nj += 1
                        for j in chunks[ci:nj]:
                            cw = min(128, NIJ - j * 128)
                            ps = ps_sim.tile([128, HW], F32, tag="ps_sim")
                            nc.tensor.matmul(
                                ps[:cw, :],
                                lhsT=k_bf[:, j * 128 : j * 128 + cw],
                                rhs=qq_bf[:, b * HW : (b + 1) * HW],
                                start=True,
                                stop=True,
                            )
                            e = etp.tile([128, HW], BF16, tag="et")
                            if cw < 128:
                                nc.vector.memset(e, 0.0)
                            nc.scalar.activation(
                                out=e[:cw, :],
                                in_=ps[:cw, :],
                                func=mybir.ActivationFunctionType.Exp,
                            )
                            et[j] = e
                        for j in chunks[ci:nj]:
                            cw = min(128, NIJ - j * 128)
                            vt = vt1p.tile([128, 132], BF16, tag="vt1")
                            if cw < 128:
                                nc.vector.memset(vt, 0.0)
                            pt = ps_vt.tile([128, 128], BF16, tag="ps_vt")
                            nc.tensor.transpose(
                                pt[:cw, :],
                                v_bf[:, j * 128 : j * 128 + cw],
                                id_bf,
                            )
                            nc.vector.tensor_copy(vt[:cw, 0:128], pt[:cw, :])
                            nc.vector.memset(vt[:, 128:132], 1.0)
                            vt1[j] = vt
                        ci = nj

                    # PV: accumulate ctx_raw for both hw-halves
                    for j in PIECE_CHUNKS[p]:
                        for h in range(2):
                            hww = 128 if h == 0 else HW - 128
                            nc.tensor.matmul(
                                pc[h][:hww, 0:132],
                                lhsT=et[j][:, h * 128 : h * 128 + hww],
                                rhs=vt1[j][:, 0:132],
                                start=(j == 0),
                                stop=(j == NCH - 1),
                            )

                for h in range(2):
                    hww = 128 if h == 0 else HW - 128
                    r = small.tile([128, 1], F32, tag="recip")
                    nc.vector.reciprocal(r[:hww], pc[h][:hww, 128:129])
                    ctx = small.tile([128, 128], F32, tag="ctx")
                    nc.vector.tensor_scalar_mul(
                        ctx[:hww, :], pc[h][:hww, 0:128], r[:hww]
                    )
                    d = small.tile([128, 128], F32, tag="diff")
                    nc.vector.tensor_sub(
                        d[:hww, :], qvT[(b, h)][:hww, :], ctx[:hww, :]
                    )
                    d2 = small.tile([128, 128], F32, tag="d2")
                    nc.vector.tensor_mul(d2[:hww, :], d[:hww, :], d[:hww, :])
                    nc.vector.reduce_sum(
                        partials[:hww, 2 * b + h : 2 * b + h + 1],
                        d2[:hww, :],
                        axis=mybir.AxisListType.X,
                    )

            # ---- final reduction to scalar ----
            tot = small.tile([128, 1], F32, tag="tot")
            nc.vector.reduce_sum(tot, partials, axis=mybir.AxisListType.X)
            ones = small.tile([128, 1], F32, tag="ones")
            nc.vector.memset(ones, 1.0)
            pf = ps_vt.tile([128, 128], F32, tag="ps_vt")
            nc.tensor.matmul(pf[0:1, 0:1], lhsT=tot, rhs=ones, start=True, stop=True)
            ob = small.tile([1, 1], F32, tag="ob")
            nc.vector.tensor_copy(ob, pf[0:1, 0:1])
            nc.sync.dma_start(out=out_d, in_=ob)

    nc.compile()
    return nc


_NC = None


def kernel(query_repr, supports_repr, W_qk, W_v):
    global _NC
    q = np.ascontiguousarray(np.asarray(query_repr, dtype=np.float32)).reshape(
        32, C, HW
    )
    s = np.ascontiguousarray(np.asarray(supports_repr, dtype=np.float32)).reshape(
        32, N_SUP, C, HW
    )
    wqk = np.ascontiguousarray(np.asarray(W_qk, dtype=np.float32))
    wv = np.ascontiguousarray(np.asarray(W_v, dtype=np.float32))

    if _NC is None:
        _NC = build_bass()

    in_maps = []
    for core in range(8):
        b0 = core * B_PER_CORE
        in_maps.append(
            {
                "q": np.ascontiguousarray(q[b0 : b0 + B_PER_CORE]),
                "s": np.ascontiguousarray(s[b0 : b0 + B_PER_CORE]),
                "wqk": wqk,
                "wv": wv,
            }
        )
    res = run_bass_kernel_spmd(
        _NC, in_maps, core_ids=list(range(8)),
        trace=bool(int(os.environ.get("KTRACE", "0"))),
    )
    total = sum(float(r["out"][0, 0]) for r in res.results) / float(HW)
    kernel._last_results = res
    return np.asarray(total, dtype=np.float32)


# revision 22
# speedup vs baseline: 1.0559x; 1.0279x over previous
"""CrossTransformer kernel for Trainium2, data-parallel over batch across 8 cores.

Math per batch b (B=32, N=25, C=512, H=W=14, DK=DV=128):
  qq = Wqk @ Q    [128, 196]      qv = Wv @ Q     [128, 196]
  K  = Wqk @ S    [128, 4900]     V  = Wv @ S     [128, 4900]
  simT[nij, hw] = K^T @ qq        (computed directly in transposed layout)
  E = exp(simT)                   (no max subtraction; |sim| <~ 60 is safe in fp32)
  ctx_raw[hw, v+1] = sum_nij E[nij, hw]^T @ [V^T | 1]   (ones column -> softmax denom)
  ctx = ctx_raw[:, :128] / ctx_raw[:, 128:129]
  partial += sum((qv^T - ctx)^2)
Output per core: scalar partial sum over its 4 batches; host sums and divides by H*W.

S is streamed with one DMA per (batch, c-chunk, n-half): 8 transfers of
~1.2-1.3 MB per batch (784B contiguous runs), instead of many ~100-250KB
pieces - the descriptor-generation path (HWDGE, ~625ns/DMA) serializes all
DMAs, so transfer count, not just bytes, bounds the kernel.
"""

import os
import sys

sys.path.insert(0, "/opt/trn_rl_repo")

import numpy as np

import concourse.bass as bass
import concourse.bacc as bacc
import concourse.mybir as mybir
import concourse.tile as tile
from concourse.bass_utils import run_bass_kernel_spmd
from concourse.masks import make_identity

F32 = mybir.dt.float32
F32R = mybir.dt.float32r
BF16 = mybir.dt.bfloat16

B_PER_CORE = 4
N_SUP = 25
C = 512
HW = 196
NIJ = N_SUP * HW  # 4900
DK = 128
NCH = (NIJ + 127) // 128  # 39 chunks of <=128 along nij
CCH = C // 128  # 4 c-chunks
NPIECES = [(0, 6), (6, 12), (12, 18), (18, 23), (23, N_SUP)]  # one DMA per (b, cc, piece)
SMAX = max(n1 - n0 for n0, n1 in NPIECES) * HW  # 1176: widest piece tile


def _piece_chunks():
    """Assign each 128-wide nij chunk to the first piece whose data covers it."""
    out = [[] for _ in NPIECES]
    ends = [n1 * HW for _, n1 in NPIECES]
    for j in range(NCH):
        hi = min((j + 1) * 128, NIJ)
        p = next(i for i, e in enumerate(ends) if hi <= e)
        out[p].append(j)
    return out


PIECE_CHUNKS = _piece_chunks()


def _proj_widths(w):
    """Split w into <=512-wide even tiles >=256 (fp32r full-rate, ISA-legal)."""
    nt = (w + 511) // 512
    base = (w // (2 * nt)) * 2
    ws = [base + 2] * ((w - nt * base) // 2)
    ws += [base] * (nt - len(ws))
    return ws


def build_bass():
    nc = bacc.Bacc(
        "TRN2", target_bir_lowering=False, debug=False, enable_asserts=False
    )
    q_d = nc.dram_tensor("q", [B_PER_CORE, C, HW], F32, kind="ExternalInput").ap()
    s_d = nc.dram_tensor(
        "s", [B_PER_CORE, N_SUP, C, HW], F32, kind="ExternalInput"
    ).ap()
    wqk_d = nc.dram_tensor("wqk", [DK, C], F32, kind="ExternalInput").ap()
    wv_d = nc.dram_tensor("wv", [DK, C], F32, kind="ExternalInput").ap()
    out_d = nc.dram_tensor("out", [1, 1], F32, kind="ExternalOutput").ap()

    with tile.TileContext(nc) as tc:
        with (
            tc.tile_pool(name="const", bufs=1) as const,
            tc.tile_pool(name="spool", bufs=22) as spool,
            tc.tile_pool(name="kvbf", bufs=2) as kvbf,
            tc.tile_pool(name="vt1p", bufs=20) as vt1p,
            tc.tile_pool(name="etp", bufs=20) as etp,
            tc.tile_pool(name="small", bufs=4) as small,
            tc.tile_pool(name="ps_proj", bufs=2, space="PSUM") as ps_proj,
            tc.tile_pool(name="ps_sim", bufs=2, space="PSUM") as ps_sim,
            tc.tile_pool(name="ps_vt", bufs=2, space="PSUM") as ps_vt,
            tc.tile_pool(name="ps_ctx", bufs=1, space="PSUM") as ps_ctx,
        ):
            # ---- constants / weights ----
            id_f32 = const.tile([128, 128], F32, tag="id_f32")
            make_identity(nc, id_f32)
            id_bf = const.tile([128, 128], BF16, tag="id_bf")
            make_identity(nc, id_bf)

            wqk_sb = const.tile([128, C], F32, tag="wqk_sb")
            nc.sync.dma_start(out=wqk_sb, in_=wqk_d)
            wv_sb = const.tile([128, C], F32, tag="wv_sb")
            nc.sync.dma_start(out=wv_sb, in_=wv_d)

            wqkT = []
            wvT = []
            for cc in range(CCH):
                for (src, dstl, nm) in ((wqk_sb, wqkT, "qk"), (wv_sb, wvT, "v")):
                    pt = ps_vt.tile([128, 128], F32, tag="ps_vt")
                    nc.tensor.transpose(pt, src[:, cc * 128 : (cc + 1) * 128], id_f32)
                    wt = const.tile([128, 128], F32R, tag=f"w{nm}T{cc}")
                    nc.vector.tensor_copy(wt, pt)
                    dstl.append(wt)

            # ---- query load + projections (all 4 batches at once) ----
            qsb = []
            for cc in range(CCH):
                qt = spool.tile([128, B_PER_CORE * HW], F32R, tag="s_t")
                src = q_d[:, cc * 128 : (cc + 1) * 128, :].rearrange(
                    "b c ij -> c b ij"
                ).bitcast(F32R)
                nc.sync.dma_start(
                    out=qt.rearrange("p (b ij) -> p b ij", b=B_PER_CORE), in_=src
                )
                qsb.append(qt)

            qq_bf = const.tile([128, B_PER_CORE * HW], BF16, tag="qq_bf")
            qv_sb = const.tile([128, B_PER_CORE * HW], F32, tag="qv_sb")
            for wT, dst in ((wqkT, qq_bf), (wvT, qv_sb)):
                for half in range(2):
                    hw0 = half * 392
                    pq = ps_proj.tile([128, 512], F32, tag="ps_proj")
                    for cc in range(CCH):
                        nc.tensor.matmul(
                            pq[:, :392],
                            lhsT=wT[cc],
                            rhs=qsb[cc][:, hw0 : hw0 + 392],
                            start=(cc == 0),
                            stop=(cc == CCH - 1),
                        )
                    nc.vector.tensor_copy(dst[:, hw0 : hw0 + 392], pq[:, :392])

            # qv^T per (b, hw-chunk): [hw<=128, 128] fp32 — matches ctx layout
            qvT = {}
            for b in range(B_PER_CORE):
                for h in range(2):
                    hww = 128 if h == 0 else HW - 128
                    pt = ps_vt.tile([128, 128], F32, tag="ps_vt")
                    nc.tensor.transpose(
                        pt[:hww, :],
                        qv_sb[:, b * HW + h * 128 : b * HW + h * 128 + hww],
                        id_f32,
                    )
                    qt = const.tile([128, 128], F32, tag=f"qvT{b}_{h}")
                    nc.vector.tensor_copy(qt[:hww, :], pt[:hww, :])
                    qvT[(b, h)] = qt

            partials = const.tile([128, 2 * B_PER_CORE], F32, tag="partials")
            nc.vector.memset(partials, 0.0)

            # ---- main pipeline: one stream of (batch, n-piece) units ----
            # Per piece: DMA 4 c-slabs -> K/V proj per tile -> sim+exp +
            # V^T transposes for the covered chunks. The piece's PV
            # accumulation is software-pipelined one piece later (emitted
            # after the NEXT piece's tiles), so PV and its wait on the
            # serial ACT exp chain fill PE idle time instead of extending
            # each piece's critical chain. Finalize(b) follows PV of b's
            # last piece, wherever that lands in the stream.
            def emit_pv(pend):
                pb, ppc, pet, pvt1, pchunks = pend
                for j in pchunks:
                    for h in range(2):
                        hww = 128 if h == 0 else HW - 128
                        nc.tensor.matmul(
                            ppc[h][:hww, 0:132],
                            lhsT=pet[j][:, h * 128 : h * 128 + hww],
                            rhs=pvt1[j][:, 0:132],
                            start=(j == 0),
                            stop=(j == NCH - 1),
                        )
                if pchunks[-1] == NCH - 1:
                    for h in range(2):
                        hww = 128 if h == 0 else HW - 128
                        r = small.tile([128, 1], F32, tag="recip")
                        nc.vector.reciprocal(r[:hww], ppc[h][:hww, 128:129])
                        ctx = small.tile([128, 128], F32, tag="ctx")
                        nc.vector.tensor_scalar_mul(
                            ctx[:hww, :], ppc[h][:hww, 0:128], r[:hww]
                        )
                        d = small.tile([128, 128], F32, tag="diff")
                        nc.vector.tensor_sub(
                            d[:hww, :], qvT[(pb, h)][:hww, :], ctx[:hww, :]
                        )
                        d2 = small.tile([128, 128], F32, tag="d2")
                        nc.vector.tensor_mul(d2[:hww, :], d[:hww, :], d[:hww, :])
                        nc.vector.reduce_sum(
                            partials[:hww, 2 * pb + h : 2 * pb + h + 1],
                            d2[:hww, :],
                            axis=mybir.AxisListType.X,
                        )

            pending = None
            k_bf = v_bf = pc = None
            for b in range(B_PER_CORE):
                for p, (n0, n1) in enumerate(NPIECES):
                    if p == 0:
                        k_bf = kvbf.tile([128, NIJ], BF16, tag="k_bf")
                        v_bf = kvbf.tile([128, NIJ], BF16, tag="v_bf")
                        # one PSUM bank per hw-half: start=True clears
                        # has_written BANK-wide, so the two accumulation
                        # groups must not share a bank.
                        pc = [
                            ps_ctx.tile([128, 132], F32, tag="pc0", name="pc0"),
                            ps_ctx.tile([128, 132], F32, tag="pc1", name="pc1"),
                        ]
                    nn = n1 - n0
                    w = nn * HW
                    off0 = n0 * HW
                    st = []
                    for cc in range(CCH):
                        s_t = spool.tile([128, SMAX], F32R, tag="s_t")
                        src = s_d[
                            b, n0:n1, cc * 128 : (cc + 1) * 128, :
                        ].rearrange("n c ij -> c n ij").bitcast(F32R)
                        nc.sync.dma_start(
                            out=s_t[:, 0:w].rearrange(
                                "p (n ij) -> p n ij", n=nn
                            ),
                            in_=src,
                        )
                        st.append(s_t)
                    # Per proj tile: V matmuls (copy on ACT, hidden under the
                    # K matmuls), K matmuls (copy on DVE), then sim+exp and
                    # V^T transposes for every chunk the copies now cover —
                    # this starts the serial ACT exp chain as early as
                    # possible instead of after the whole piece's projections.
                    et = {}
                    vt1 = {}
                    chunks = PIECE_CHUNKS[p]
                    ci = 0
                    t0 = 0
                    for tw in _proj_widths(w):
                        pv = ps_proj.tile([128, 512], F32, tag="ps_proj")
                        for cc in range(CCH):
                            nc.tensor.matmul(
                                pv[:, :tw],
                                lhsT=wvT[cc],
                                rhs=st[cc][:, t0 : t0 + tw],
                                start=(cc == 0),
                                stop=(cc == CCH - 1),
                            )
                        nc.scalar.copy(
                            v_bf[:, off0 + t0 : off0 + t0 + tw], pv[:, :tw]
                        )
                        pk = ps_proj.tile([128, 512], F32, tag="ps_proj")
                        for cc in range(CCH):
                            nc.tensor.matmul(
                                pk[:, :tw],
                                lhsT=wqkT[cc],
                                rhs=st[cc][:, t0 : t0 + tw],
                                start=(cc == 0),
                                stop=(cc == CCH - 1),
                            )
                        nc.vector.tensor_copy(
                            k_bf[:, off0 + t0 : off0 + t0 + tw], pk[:, :tw]
                        )
                        t0 += tw
                        covered = off0 + t0
                        nj = ci
                        while (
                            nj < len(chunks)
                            and min((chunks[nj] + 1) * 128, NIJ) <= covered
                        ):
                            nj += 1
                        for j in chunks[ci:nj]:
                            cw = min(128, NIJ - j * 128)
                            ps = ps_sim.tile([128, HW], F32, tag="ps_sim")
                            nc.tensor.matmul(
                                ps[:cw, :],
                                lhsT=k_bf[:, j * 128 : j * 128 + cw],
                                rhs=qq_bf[:, b * HW : (b + 1) * HW],
                                start=True,
                                stop=True,
                            )
                            e = etp.tile([128, HW], BF16, tag="et")
                            if cw < 128:
                                nc.vector.memset(e, 0.0)
                            nc.scalar.activation(
                                out=e[:cw, :],
                                in_=ps[:cw, :],
                                func=mybir.ActivationFunctionType.Exp,
                            )
                            et[j] = e
                        for j in chunks[ci:nj]:
                            cw = min(128, NIJ - j * 128)
                            vt = vt1p.tile([128, 132], BF16, tag="vt1")
                            if cw < 128:
                                nc.vector.memset(vt, 0.0)
                            pt = ps_vt.tile([128, 128], BF16, tag="ps_vt")
                            nc.tensor.transpose(
                                pt[:cw, :],
                                v_bf[:, j * 128 : j * 128 + cw],
                                id_bf,
                            )
                            nc.vector.tensor_copy(vt[:cw, 0:128], pt[:cw, :])
                            nc.vector.memset(vt[:, 128:132], 1.0)
                            vt1[j] = vt
                        ci = nj

                    if pending is not None:
                        emit_pv(pending)
                    pending = (b, pc, et, vt1, chunks)
            emit_pv(pending)

            # ---- final reduction to scalar ----
            tot = small.tile([128, 1], F32, tag="tot")
            nc.vector.reduce_sum(tot, partials, axis=mybir.AxisListType.X)
            ones = small.tile([128, 1], F32, tag="ones")
            nc.vector.memset(ones, 1.0)
            pf = ps_vt.tile([128, 128], F32, tag="ps_vt")
            nc.tensor.matmul(pf[0:1, 0:1], lhsT=tot, rhs=ones, start=True, stop=True)
            ob = small.tile([1, 1], F32, tag="ob")
            nc.vector.tensor_copy(ob, pf[0:1, 0:1])
            nc.sync.dma_start(out=out_d, in_=ob)

    nc.compile()
    return nc


_NC = None


def kernel(query_repr, supports_repr, W_qk, W_v):
    global _NC
    q = np.ascontiguousarray(np.asarray(query_repr, dtype=np.float32)).reshape(
        32, C, HW
    )
    s = np.ascontiguousarray(np.asarray(supports_repr, dtype=np.float32)).reshape(
        32, N_SUP, C, HW
    )
    wqk = np.ascontiguousarray(np.asarray(W_qk, dtype=np.float32))
    wv = np.ascontiguousarray(np.asarray(W_v, dtype=np.float32))

    if _NC is None:
        _NC = build_bass()

    in_maps = []
    for core in range(8):
        b0 = core * B_PER_CORE
        in_maps.append(
            {
                "q": np.ascontiguousarray(q[b0 : b0 + B_PER_CORE]),
                "s": np.ascontiguousarray(s[b0 : b0 + B_PER_CORE]),
                "wqk": wqk,
                "wv": wv,
            }
        )
    res = run_bass_kernel_spmd(
        _NC, in_maps, core_ids=list(range(8)),
        trace=bool(int(os.environ.get("KTRACE", "0"))),
    )
    total = sum(float(r["out"][0, 0]) for r in res.results) / float(HW)
    kernel._last_results = res
    return np.asarray(total, dtype=np.float32)


# revision 26
# speedup vs baseline: 1.0848x; 1.0274x over previous
"""CrossTransformer kernel for Trainium2, data-parallel over batch across 8 cores.

Math per batch b (B=32, N=25, C=512, H=W=14, DK=DV=128):
  qq = Wqk @ Q    [128, 196]      qv = Wv @ Q     [128, 196]
  K  = Wqk @ S    [128, 4900]     V  = Wv @ S     [128, 4900]
  simT[nij, hw] = K^T @ qq        (computed directly in transposed layout)
  E = exp(simT)                   (no max subtraction; |sim| <~ 60 is safe in fp32)
  ctx_raw[hw, v+1] = sum_nij E[nij, hw]^T @ [V^T | 1]   (ones column -> softmax denom)
  ctx = ctx_raw[:, :128] / ctx_raw[:, 128:129]
  partial += sum((qv^T - ctx)^2)
Output per core: scalar partial sum over its 4 batches; host sums and divides by H*W.

S is streamed with one DMA per (batch, c-chunk, n-half): 8 transfers of
~1.2-1.3 MB per batch (784B contiguous runs), instead of many ~100-250KB
pieces - the descriptor-generation path (HWDGE, ~625ns/DMA) serializes all
DMAs, so transfer count, not just bytes, bounds the kernel.
"""

import os
import sys

sys.path.insert(0, "/opt/trn_rl_repo")

import numpy as np

import concourse.bass as bass
import concourse.bacc as bacc
import concourse.mybir as mybir
import concourse.tile as tile
from concourse.bass_utils import run_bass_kernel_spmd
from concourse.masks import make_identity

F32 = mybir.dt.float32
F32R = mybir.dt.float32r
BF16 = mybir.dt.bfloat16

B_PER_CORE = 4
N_SUP = 25
C = 512
HW = 196
NIJ = N_SUP * HW  # 4900
DK = 128
NCH = (NIJ + 127) // 128  # 39 chunks of <=128 along nij
CCH = C // 128  # 4 c-chunks
NPIECES = [(0, 6), (6, 12), (12, 18), (18, 23), (23, N_SUP)]  # one DMA per (b, cc, piece)
SMAX = max(n1 - n0 for n0, n1 in NPIECES) * HW  # 1176: widest piece tile


def _piece_chunks():
    """Assign each 128-wide nij chunk to the first piece whose data covers it."""
    out = [[] for _ in NPIECES]
    ends = [n1 * HW for _, n1 in NPIECES]
    for j in range(NCH):
        hi = min((j + 1) * 128, NIJ)
        p = next(i for i, e in enumerate(ends) if hi <= e)
        out[p].append(j)
    return out


PIECE_CHUNKS = _piece_chunks()


def _proj_widths(w):
    """Split w into <=512-wide even tiles >=256 (fp32r full-rate, ISA-legal)."""
    nt = (w + 511) // 512
    base = (w // (2 * nt)) * 2
    ws = [base + 2] * ((w - nt * base) // 2)
    ws += [base] * (nt - len(ws))
    return ws


def build_bass():
    nc = bacc.Bacc(
        "TRN2", target_bir_lowering=False, debug=False, enable_asserts=False
    )
    q_d = nc.dram_tensor("q", [B_PER_CORE, C, HW], F32, kind="ExternalInput").ap()
    s_d = nc.dram_tensor(
        "s", [B_PER_CORE, N_SUP, C, HW], F32, kind="ExternalInput"
    ).ap()
    wqk_d = nc.dram_tensor("wqk", [DK, C], F32, kind="ExternalInput").ap()
    wv_d = nc.dram_tensor("wv", [DK, C], F32, kind="ExternalInput").ap()
    out_d = nc.dram_tensor("out", [1, 1], F32, kind="ExternalOutput").ap()

    with tile.TileContext(nc) as tc:
        with (
            tc.tile_pool(name="const", bufs=1) as const,
            tc.tile_pool(name="spool", bufs=22) as spool,
            tc.tile_pool(name="kvbf", bufs=2) as kvbf,
            tc.tile_pool(name="vt1p", bufs=20) as vt1p,
            tc.tile_pool(name="etp", bufs=20) as etp,
            tc.tile_pool(name="small", bufs=4) as small,
            tc.tile_pool(name="ps_proj", bufs=3, space="PSUM") as ps_proj,
            tc.tile_pool(name="ps_sim", bufs=2, space="PSUM") as ps_sim,
            tc.tile_pool(name="ps_vt", bufs=2, space="PSUM") as ps_vt,
            tc.tile_pool(name="ps_ctx", bufs=1, space="PSUM") as ps_ctx,
        ):
            # ---- constants / weights ----
            id_f32 = const.tile([128, 128], F32, tag="id_f32")
            make_identity(nc, id_f32)
            id_bf = const.tile([128, 128], BF16, tag="id_bf")
            make_identity(nc, id_bf)

            wqk_sb = const.tile([128, C], F32, tag="wqk_sb")
            nc.sync.dma_start(out=wqk_sb, in_=wqk_d)
            wv_sb = const.tile([128, C], F32, tag="wv_sb")
            nc.sync.dma_start(out=wv_sb, in_=wv_d)

            wqkT = []
            wvT = []
            for cc in range(CCH):
                for (src, dstl, nm) in ((wqk_sb, wqkT, "qk"), (wv_sb, wvT, "v")):
                    pt = ps_vt.tile([128, 128], F32, tag="ps_vt")
                    nc.tensor.transpose(pt, src[:, cc * 128 : (cc + 1) * 128], id_f32)
                    wt = const.tile([128, 128], F32R, tag=f"w{nm}T{cc}")
                    nc.vector.tensor_copy(wt, pt)
                    dstl.append(wt)

            # ---- query load + projections (all 4 batches at once) ----
            qsb = []
            for cc in range(CCH):
                qt = spool.tile([128, B_PER_CORE * HW], F32R, tag="s_t")
                src = q_d[:, cc * 128 : (cc + 1) * 128, :].rearrange(
                    "b c ij -> c b ij"
                ).bitcast(F32R)
                nc.sync.dma_start(
                    out=qt.rearrange("p (b ij) -> p b ij", b=B_PER_CORE), in_=src
                )
                qsb.append(qt)

            qq_bf = const.tile([128, B_PER_CORE * HW], BF16, tag="qq_bf")
            qv_sb = const.tile([128, B_PER_CORE * HW], F32, tag="qv_sb")
            for wT, dst in ((wqkT, qq_bf), (wvT, qv_sb)):
                for half in range(2):
                    hw0 = half * 392
                    pq = ps_proj.tile([128, 512], F32, tag="ps_proj")
                    for cc in range(CCH):
                        nc.tensor.matmul(
                            pq[:, :392],
                            lhsT=wT[cc],
                            rhs=qsb[cc][:, hw0 : hw0 + 392],
                            start=(cc == 0),
                            stop=(cc == CCH - 1),
                        )
                    nc.vector.tensor_copy(dst[:, hw0 : hw0 + 392], pq[:, :392])

            # qv^T per (b, hw-chunk): [hw<=128, 128] fp32 — matches ctx layout
            qvT = {}
            for b in range(B_PER_CORE):
                for h in range(2):
                    hww = 128 if h == 0 else HW - 128
                    pt = ps_vt.tile([128, 128], F32, tag="ps_vt")
                    nc.tensor.transpose(
                        pt[:hww, :],
                        qv_sb[:, b * HW + h * 128 : b * HW + h * 128 + hww],
                        id_f32,
                    )
                    qt = const.tile([128, 128], F32, tag=f"qvT{b}_{h}")
                    nc.vector.tensor_copy(qt[:hww, :], pt[:hww, :])
                    qvT[(b, h)] = qt

            partials = const.tile([128, 2 * B_PER_CORE], F32, tag="partials")
            nc.vector.memset(partials, 0.0)

            # ---- main pipeline: one stream of (batch, n-piece) units ----
            # Per piece: DMA 4 c-slabs -> K/V proj per tile -> sim+exp +
            # V^T transposes for the covered chunks. The piece's PV
            # accumulation is software-pipelined one piece later (emitted
            # after the NEXT piece's tiles), so PV and its wait on the
            # serial ACT exp chain fill PE idle time instead of extending
            # each piece's critical chain. Finalize(b) follows PV of b's
            # last piece, wherever that lands in the stream.
            clr = const.tile([128, 1], F32, tag="clr")
            nc.vector.memset(clr, 0.0)

            def emit_pv(pend):
                # PV matmuls never use start=True: start clears has_written
                # for the WHOLE bank (both halves live in one bank). The
                # per-batch dummy matmul on col 264 did the clear; first
                # write per element then overwrites (hw bit 0), rest add.
                pb, ppc, pet, pvt1, pchunks = pend
                for j in pchunks:
                    for h in range(2):
                        hww = 128 if h == 0 else HW - 128
                        nc.tensor.matmul(
                            ppc[:hww, h * 132 : h * 132 + 132],
                            lhsT=pet[j][:, h * 128 : h * 128 + hww],
                            rhs=pvt1[j][:, 0:132],
                            start=False,
                            stop=(j == NCH - 1 and h == 1),
                        )
                if pchunks[-1] == NCH - 1:
                    for h in range(2):
                        hww = 128 if h == 0 else HW - 128
                        r = small.tile([128, 1], F32, tag="recip")
                        nc.vector.reciprocal(
                            r[:hww], ppc[:hww, h * 132 + 128 : h * 132 + 129]
                        )
                        ctx = small.tile([128, 128], F32, tag="ctx")
                        nc.vector.tensor_scalar_mul(
                            ctx[:hww, :], ppc[:hww, h * 132 : h * 132 + 128], r[:hww]
                        )
                        d = small.tile([128, 128], F32, tag="diff")
                        nc.vector.tensor_sub(
                            d[:hww, :], qvT[(pb, h)][:hww, :], ctx[:hww, :]
                        )
                        d2 = small.tile([128, 128], F32, tag="d2")
                        nc.vector.tensor_mul(d2[:hww, :], d[:hww, :], d[:hww, :])
                        nc.vector.reduce_sum(
                            partials[:hww, 2 * pb + h : 2 * pb + h + 1],
                            d2[:hww, :],
                            axis=mybir.AxisListType.X,
                        )

            pending = None
            k_bf = v_bf = pc = None
            for b in range(B_PER_CORE):
                for p, (n0, n1) in enumerate(NPIECES):
                    if p == 0:
                        k_bf = kvbf.tile([128, NIJ], BF16, tag="k_bf")
                        v_bf = kvbf.tile([128, NIJ], BF16, tag="v_bf")
                        # both hw-halves in ONE bank (cols 0:132 / 132:264);
                        # col 264 takes the per-batch start=True dummy that
                        # clears the bank's has_written bits before PV.
                        pc = ps_ctx.tile([128, 268], F32, tag="ps_ctx")
                        # all 128 partitions: the has_written clear covers
                        # (instruction partition range) x (whole bank width)
                        nc.tensor.matmul(
                            pc[:, 264:265],
                            lhsT=id_f32,
                            rhs=clr,
                            start=True,
                            stop=True,
                        )
                    nn = n1 - n0
                    w = nn * HW
                    off0 = n0 * HW
                    st = []
                    for cc in range(CCH):
                        s_t = spool.tile([128, SMAX], F32R, tag="s_t")
                        src = s_d[
                            b, n0:n1, cc * 128 : (cc + 1) * 128, :
                        ].rearrange("n c ij -> c n ij").bitcast(F32R)
                        nc.sync.dma_start(
                            out=s_t[:, 0:w].rearrange(
                                "p (n ij) -> p n ij", n=nn
                            ),
                            in_=src,
                        )
                        st.append(s_t)
                    # Per proj tile: V matmuls (copy on ACT, hidden under the
                    # K matmuls), K matmuls (copy on DVE), then sim+exp and
                    # V^T transposes for every chunk the copies now cover —
                    # this starts the serial ACT exp chain as early as
                    # possible instead of after the whole piece's projections.
                    et = {}
                    vt1 = {}
                    chunks = PIECE_CHUNKS[p]
                    ci = 0
                    t0 = 0
                    for tw in _proj_widths(w):
                        pv = ps_proj.tile([128, 512], F32, tag="ps_proj")
                        for cc in range(CCH):
                            nc.tensor.matmul(
                                pv[:, :tw],
                                lhsT=wvT[cc],
                                rhs=st[cc][:, t0 : t0 + tw],
                                start=(cc == 0),
                                stop=(cc == CCH - 1),
                            )
                        nc.scalar.copy(
                            v_bf[:, off0 + t0 : off0 + t0 + tw], pv[:, :tw]
                        )
                        pk = ps_proj.tile([128, 512], F32, tag="ps_proj")
                        for cc in range(CCH):
                            nc.tensor.matmul(
                                pk[:, :tw],
                                lhsT=wqkT[cc],
                                rhs=st[cc][:, t0 : t0 + tw],
                                start=(cc == 0),
                                stop=(cc == CCH - 1),
                            )
                        nc.vector.tensor_copy(
                            k_bf[:, off0 + t0 : off0 + t0 + tw], pk[:, :tw]
                        )
                        t0 += tw
                        covered = off0 + t0
                        nj = ci
                        while (
                            nj < len(chunks)
                            and min((chunks[nj] + 1) * 128, NIJ) <= covered
                        ):
                            nj += 1
                        for j in chunks[ci:nj]:
                            cw = min(128, NIJ - j * 128)
                            ps = ps_sim.tile([128, HW], F32, tag="ps_sim")
                            nc.tensor.matmul(
                                ps[:cw, :],
                                lhsT=k_bf[:, j * 128 : j * 128 + cw],
                                rhs=qq_bf[:, b * HW : (b + 1) * HW],
                                start=True,
                                stop=True,
                            )
                            e = etp.tile([128, HW], BF16, tag="et")
                            if cw < 128:
                                nc.vector.memset(e, 0.0)
                            nc.scalar.activation(
                                out=e[:cw, :],
                                in_=ps[:cw, :],
                                func=mybir.ActivationFunctionType.Exp,
                            )
                            et[j] = e
                        for j in chunks[ci:nj]:
                            cw = min(128, NIJ - j * 128)
                            vt = vt1p.tile([128, 132], BF16, tag="vt1")
                            if cw < 128:
                                nc.vector.memset(vt, 0.0)
                            pt = ps_vt.tile([128, 128], BF16, tag="ps_vt")
                            nc.tensor.transpose(
                                pt[:cw, :],
                                v_bf[:, j * 128 : j * 128 + cw],
                                id_bf,
                            )
                            nc.vector.tensor_copy(vt[:cw, 0:128], pt[:cw, :])
                            nc.vector.memset(vt[:, 128:132], 1.0)
                            vt1[j] = vt
                        ci = nj

                    if pending is not None:
                        emit_pv(pending)
                    pending = (b, pc, et, vt1, chunks)
            emit_pv(pending)

            # ---- final reduction to scalar ----
            tot = small.tile([128, 1], F32, tag="tot")
            nc.vector.reduce_sum(tot, partials, axis=mybir.AxisListType.X)
            ones = small.tile([128, 1], F32, tag="ones")
            nc.vector.memset(ones, 1.0)
            pf = ps_vt.tile([128, 128], F32, tag="ps_vt")
            nc.tensor.matmul(pf[0:1, 0:1], lhsT=tot, rhs=ones, start=True, stop=True)
            ob = small.tile([1, 1], F32, tag="ob")
            nc.vector.tensor_copy(ob, pf[0:1, 0:1])
            nc.sync.dma_start(out=out_d, in_=ob)

    nc.compile()
    return nc


_NC = None


def kernel(query_repr, supports_repr, W_qk, W_v):
    global _NC
    q = np.ascontiguousarray(np.asarray(query_repr, dtype=np.float32)).reshape(
        32, C, HW
    )
    s = np.ascontiguousarray(np.asarray(supports_repr, dtype=np.float32)).reshape(
        32, N_SUP, C, HW
    )
    wqk = np.ascontiguousarray(np.asarray(W_qk, dtype=np.float32))
    wv = np.ascontiguousarray(np.asarray(W_v, dtype=np.float32))

    if _NC is None:
        _NC = build_bass()

    in_maps = []
    for core in range(8):
        b0 = core * B_PER_CORE
        in_maps.append(
            {
                "q": np.ascontiguousarray(q[b0 : b0 + B_PER_CORE]),
                "s": np.ascontiguousarray(s[b0 : b0 + B_PER_CORE]),
                "wqk": wqk,
                "wv": wv,
            }
        )
    res = run_bass_kernel_spmd(
        _NC, in_maps, core_ids=list(range(8)),
        trace=bool(int(os.environ.get("KTRACE", "0"))),
    )
    total = sum(float(r["out"][0, 0]) for r in res.results) / float(HW)
    kernel._last_results = res
    return np.asarray(total, dtype=np.float32)
